# revision 8
# baseline (speedup 1.0000x reference)
"""Causal self-attention Trainium2 kernel (B=4, T=4096, C=384, H=6).

Sharding: 8 cores = 4 batches x 2 head-groups (3 heads each). Each core
computes y_partial = attn(x[b], heads hg) @ w_proj[rows of hg]; the host
sums the two partials per batch (the "all-reduce after c_proj" done on
host during unshard).
"""

import numpy as np
from contextlib import ExitStack

import concourse.bass as bass
import concourse.tile as tile
from concourse import mybir
from concourse.bass_utils import run_bass_kernel_spmd
from concourse.masks import make_identity
from concourse.vector_clock import ScopedClock

F32 = mybir.dt.float32
BF16 = mybir.dt.bfloat16
EXP = mybir.ActivationFunctionType.Exp
MULT = mybir.AluOpType.mult

B, T, C, H, D = 4, 4096, 384, 6, 64
HPC = 3            # heads per core
QT = 512           # q tile
KC = 128           # key chunk
SCALE = 1.0 / 8.0  # 1/sqrt(64)


# ---------------------------------------------------------------------------
# Workaround: neuronxcc CoreV3 rejects >2 sem waits on the Tile tail drain.
# Split the drain's waits into individual sync-engine wait instructions.
def _drain_and_barrier_split(self, tick_clock, wait_clock):
    nc = self.nc
    drain_inst = nc.sync.drain()
    wait_clock.add_sem_waits(
        drain_inst.ins, ScopedClock({None: tick_clock.global_clock})
    )
    si = drain_inst.ins.sync_info
    if si is not None and si.on_wait and len(si.on_wait) > 1:
        waits = list(si.on_wait)
        si.on_wait = []
        allocated = {h.name: h for h in self.sems.allocated().values()}
        for w in waits:
            h = allocated.get(w.ant_name)
            assert h is not None, f"no sem handle for drain wait {w.ant_name}"
            assert w.wait_mode == "sem-ge-imm", w.wait_mode
            nc.sync.wait_ge(h, w.wait_value)
    nc.all_engine_barrier()
    assert self.sems is not None
    popped = nc._tile_sem_poison_stack.pop()
    assert popped is self._sem_poison
    nc.clear_and_free_semaphores(list(self.sems.allocated().values()))
    nc.all_engine_barrier()


tile.TileContext._drain_and_barrier = _drain_and_barrier_split


MAX_WAITS = 1  # CoreV3 per-instruction sem-wait capacity (S3_LW holds only 1)


def _split_excess_waits(nc):
    """Hoist sem waits beyond MAX_WAITS onto same-engine NOPs inserted
    directly before the over-limit instruction (waits are order-free)."""
    for fn in nc.m.functions:
        for bb in fn.blocks:
            insts = list(bb.instructions)
            out = []
            changed = False
            for inst in insts:
                si = inst.sync_info
                if si is not None and si.on_wait and len(si.on_wait) > MAX_WAITS:
                    waits = list(si.on_wait)
                    excess, keep = waits[:-MAX_WAITS], waits[-MAX_WAITS:]
                    si.on_wait = keep
                    inst.sync_info = si
                    for i in range(0, len(excess), MAX_WAITS):
                        nop = mybir.InstNoOp(
                            name=f"{inst.name}-waitsplit-{i}", ins=[], outs=[]
                        )
                        nop.engine = inst.engine
                        nop.sync_info = mybir.SyncInfo(
                            on_wait=excess[i:i + MAX_WAITS], on_update=[]
                        )
                        nc.register_instruction(nop)
                        out.append(nop)
                    changed = True
                out.append(inst)
            if changed:
                bb.instructions = out
# ---------------------------------------------------------------------------


def build(t=T):
    nqt = t // QT          # q tiles
    nkc = t // KC          # key chunks
    ntb = t // QT          # token blocks for phase A (512 tokens each)

    nc = bass.Bass()
    x_d = nc.dram_tensor("x", [t, C], F32, kind="ExternalInput")
    wq01_d = nc.dram_tensor("w_q01", [3, 128, 128], BF16, kind="ExternalInput")
    wk01_d = nc.dram_tensor("w_k01", [3, 128, 128], BF16, kind="ExternalInput")
    wq2_d = nc.dram_tensor("w_q2", [3, 128, 64], BF16, kind="ExternalInput")
    wk2_d = nc.dram_tensor("w_k2", [3, 128, 64], BF16, kind="ExternalInput")
    wv_d = nc.dram_tensor("w_v", [3, 128, 192], BF16, kind="ExternalInput")
    wo_d = nc.dram_tensor("w_o", [3, 64, 384], BF16, kind="ExternalInput")
    mask_d = nc.dram_tensor("masks", [4, 128, QT], BF16, kind="ExternalInput")
    y_d = nc.dram_tensor("y", [t, C], F32, kind="ExternalOutput")

    with tile.TileContext(nc) as tc, ExitStack() as ctx:
        persist = ctx.enter_context(tc.tile_pool(name="persist", bufs=1))

        # weights / masks / identity
        wq01 = persist.tile([128, 3, 128], BF16)
        wk01 = persist.tile([128, 3, 128], BF16)
        wq2 = persist.tile([128, 3, 64], BF16)
        wk2 = persist.tile([128, 3, 64], BF16)
        wv = persist.tile([128, 3, 192], BF16)
        wo = persist.tile([64, 3, 384], BF16)
        for c in range(3):
            nc.sync.dma_start(out=wq01[:, c, :], in_=wq01_d[c])
            nc.sync.dma_start(out=wk01[:, c, :], in_=wk01_d[c])
            nc.sync.dma_start(out=wq2[:, c, :], in_=wq2_d[c])
            nc.sync.dma_start(out=wk2[:, c, :], in_=wk2_d[c])
            nc.sync.dma_start(out=wv[:, c, :], in_=wv_d[c])
            nc.sync.dma_start(out=wo[:, c, :], in_=wo_d[c])
        masks = persist.tile([128, 4, QT], BF16)
        for j in range(4):
            nc.sync.dma_start(out=masks[:, j, :], in_=mask_d[j])
        ident = persist.tile([128, 128], F32)
        make_identity(nc, ident)
        ones64 = persist.tile([65, 64], BF16)  # row 64 = K=1 lhsT for bcast
        nc.vector.memset(ones64[:], 1.0)

        # persistent activations (bf16)
        qT01 = persist.tile([128, t], BF16)   # rows 0:64 h0 qT, 64:128 h1 qT
        kT01 = persist.tile([128, t], BF16)
        qT2 = persist.tile([64, t], BF16)
        kT2 = persist.tile([64, t], BF16)
        vsb = persist.tile([128, nkc, 3, 65], BF16)  # [keys, chunk, head, d|one]
        nc.vector.memset(vsb[:, :, :, 64:65], 1.0)

        # ---------------- Phase A: qkv projection -------------------------
        with (
            tc.tile_pool(name="xin", bufs=3) as xin_p,
            tc.tile_pool(name="xt", bufs=2) as xt_p,
            tc.tile_pool(name="ps_tp", bufs=4, space="PSUM") as ps_tp,
            tc.tile_pool(name="ps_qk", bufs=2, space="PSUM") as ps_qk,
            tc.tile_pool(name="ps_v", bufs=2, space="PSUM") as ps_v,
        ):
            for tb in range(ntb):
                xt = xt_p.tile([128, 3, QT], BF16)  # xT chunk block
                for s in range(4):
                    xx = xin_p.tile([128, C], F32)
                    nc.sync.dma_start(
                        out=xx[:],
                        in_=x_d[(tb * 4 + s) * 128:(tb * 4 + s + 1) * 128, :],
                    )
                    for c in range(3):
                        tp = ps_tp.tile([128, 128], F32)
                        nc.tensor.transpose(
                            tp[:], xx[:, c * 128:(c + 1) * 128], ident[:]
                        )
                        nc.vector.tensor_copy(
                            xt[:, c, s * 128:(s + 1) * 128], tp[:]
                        )
                # q/k feature-transposed blocks
                for w_sb, m, dst in (
                    (wq01, 128, qT01),
                    (wk01, 128, kT01),
                    (wq2, 64, qT2),
                    (wk2, 64, kT2),
                ):
                    ps = ps_qk.tile([128, QT], F32)
                    for c in range(3):
                        nc.tensor.matmul(
                            ps[0:m, :], w_sb[:, c, 0:m], xt[:, c, :],
                            start=(c == 0), stop=(c == 2),
                        )
                    nc.vector.tensor_copy(
                        dst[0:m, tb * QT:(tb + 1) * QT], ps[0:m, :]
                    )
                # V natural
                for s in range(4):
                    psv = ps_v.tile([128, 3, 64], F32)
                    for c in range(3):
                        nc.tensor.matmul(
                            psv[:, :, :].rearrange("p h d -> p (h d)"),
                            xt[:, c, s * 128:(s + 1) * 128],
                            wv[:, c, :],
                            start=(c == 0), stop=(c == 2),
                        )
                    nc.vector.tensor_copy(
                        vsb[:, tb * 4 + s, :, 0:64], psv[:, :, :]
                    )

        # ---------------- Phase B: attention + c_proj ---------------------
        with (
            tc.tile_pool(name="ps_s", bufs=2, space="PSUM") as ps_s,
            tc.tile_pool(name="ps_att", bufs=3, space="PSUM") as ps_att,
            tc.tile_pool(name="ps_y", bufs=1, space="PSUM") as ps_y,
            tc.tile_pool(name="pt01", bufs=1) as pt01_p,
            tc.tile_pool(name="pt2", bufs=1) as pt2_p,
            tc.tile_pool(name="attn", bufs=6) as attn_p,
            tc.tile_pool(name="linv", bufs=4) as linv_p,
            tc.tile_pool(name="bcast", bufs=4) as bcast_p,
            tc.tile_pool(name="yout", bufs=3) as yout_p,
        ):
            for qt in range(nqt):
                nch = 4 * (qt + 1)      # causal key chunks for this q tile
                q0, q1 = qt * QT, (qt + 1) * QT

                pt01 = pt01_p.tile([128, nkc, 2, QT], BF16, tag="pt01")
                pt2 = pt2_p.tile([128, nkc // 2, 2, QT], BF16, tag="pt2")

                # S^T + exp for heads 0,1 (packed via row groups 0 / 64)
                for ck in range(nch):
                    ss = ps_s.tile([128, 2, QT], F32)
                    nc.tensor.matmul(
                        ss[:, 0, :],
                        kT01[0:64, ck * KC:(ck + 1) * KC],
                        qT01[0:64, q0:q1],
                        start=True, stop=True,
                    )
                    nc.tensor.matmul(
                        ss[:, 1, :],
                        kT01[64:128, ck * KC:(ck + 1) * KC],
                        qT01[64:128, q0:q1],
                        start=True, stop=True,
                    )
                    nc.scalar.activation(
                        out=pt01[:, ck, :, :].rearrange("p h q -> p (h q)"),
                        in_=ss[:, :, :].rearrange("p h q -> p (h q)"),
                        func=EXP, scale=SCALE,
                    )
                # S^T + exp for head 2 (chunk pairs)
                for g in range(nch // 2):
                    ss = ps_s.tile([128, 2, QT], F32, tag="ss")
                    for j in range(2):
                        ck = 2 * g + j
                        nc.tensor.matmul(
                            ss[:, j, :],
                            kT2[:, ck * KC:(ck + 1) * KC],
                            qT2[:, q0:q1],
                            start=True, stop=True,
                        )
                    nc.scalar.activation(
                        out=pt2[:, g, :, :].rearrange("p h q -> p (h q)"),
                        in_=ss[:, :, :].rearrange("p h q -> p (h q)"),
                        func=EXP, scale=SCALE,
                    )

                # causal masks on the 4 diagonal chunks
                for j in range(4):
                    ck = 4 * qt + j
                    m = masks[:, j, :]
                    nc.vector.tensor_tensor(
                        out=pt01[:, ck, 0, :], in0=pt01[:, ck, 0, :], in1=m,
                        op=MULT,
                    )
                    nc.vector.tensor_tensor(
                        out=pt01[:, ck, 1, :], in0=pt01[:, ck, 1, :], in1=m,
                        op=MULT,
                    )
                    nc.vector.tensor_tensor(
                        out=pt2[:, ck // 2, ck % 2, :],
                        in0=pt2[:, ck // 2, ck % 2, :], in1=m, op=MULT,
                    )

                # att^T accumulation: lhsT = [V_h | 1]  ->  [65, QT]
                atts = []
                for h in range(3):
                    att = ps_att.tile([65, QT], F32, tag="att")
                    atts.append(att)
                    for ck in range(nch):
                        if h < 2:
                            rhs = pt01[:, ck, h, :]
                        else:
                            rhs = pt2[:, ck // 2, ck % 2, :]
                        nc.tensor.matmul(
                            att[:], vsb[:, ck, h, :], rhs,
                            start=(ck == 0), stop=(ck == nch - 1),
                        )

                # normalize: attn_h = att_h[0:64] * broadcast(1 / l_h)
                attn_tiles = []
                for h in range(3):
                    linv = linv_p.tile([65, QT], BF16, tag="linv")
                    with nc.allow_low_precision("softmax denom bf16"):
                        nc.vector.reciprocal(
                            out=linv[64:65, :], in_=atts[h][64:65, :]
                        )
                    bcp = ps_s.tile([64, QT], F32, tag="ss")
                    nc.tensor.matmul(
                        bcp[:], ones64[64:65, 0:64], linv[64:65, :],
                        start=True, stop=True,
                    )
                    bc = bcast_p.tile([64, QT], BF16, tag="bc")
                    nc.vector.tensor_copy(bc[:], bcp[:])
                    at = attn_p.tile([64, QT], BF16, tag="attn")
                    attn_tiles.append(at)
                    nc.vector.tensor_tensor(
                        out=at[:], in0=atts[h][0:64, :], in1=bc[:], op=MULT
                    )

                # c_proj: y[q0:q1] = sum_h attn_h^T @ w_o[h]
                for s in range(4):
                    yp = ps_y.tile([128, C], F32, tag="y")
                    for h in range(3):
                        nc.tensor.matmul(
                            yp[:],
                            attn_tiles[h][:, s * 128:(s + 1) * 128],
                            wo[:, h, :],
                            start=(h == 0), stop=(h == 2),
                        )
                    ysb = yout_p.tile([128, C], F32, tag="ysb")
                    nc.vector.tensor_copy(ysb[:], yp[:])
                    nc.sync.dma_start(
                        out=y_d[q0 + s * 128:q0 + (s + 1) * 128, :], in_=ysb[:]
                    )

    _split_excess_waits(nc)
    nc.finalize()
    return nc


_NC_CACHE = {}


def _get_nc(t=T):
    if t not in _NC_CACHE:
        _NC_CACHE[t] = build(t)
    return _NC_CACHE[t]


def _prep_core_inputs(x_b, w_attn, w_proj, hg, bf16):
    """Host-side shard prep for one core: batch x_b, head group hg (0/1)."""
    h0 = 3 * hg
    q = w_attn[:, 0:C]
    k = w_attn[:, C:2 * C]
    v = w_attn[:, 2 * C:3 * C]
    qcols = lambda h: q[:, h * D:(h + 1) * D]
    kcols = lambda h: k[:, h * D:(h + 1) * D]
    w_q01 = np.concatenate([qcols(h0), qcols(h0 + 1)], axis=1)      # [384,128]
    w_k01 = np.concatenate([kcols(h0), kcols(h0 + 1)], axis=1)
    w_q2 = qcols(h0 + 2)                                            # [384,64]
    w_k2 = kcols(h0 + 2)
    w_v = v[:, h0 * D:(h0 + 3) * D]                                 # [384,192]
    w_o = w_proj[h0 * D:(h0 + 3) * D, :]                            # [192,384]
    return {
        "x": np.ascontiguousarray(x_b, dtype=np.float32),
        "w_q01": np.ascontiguousarray(w_q01.reshape(3, 128, 128), dtype=bf16),
        "w_k01": np.ascontiguousarray(w_k01.reshape(3, 128, 128), dtype=bf16),
        "w_q2": np.ascontiguousarray(w_q2.reshape(3, 128, 64), dtype=bf16),
        "w_k2": np.ascontiguousarray(w_k2.reshape(3, 128, 64), dtype=bf16),
        "w_v": np.ascontiguousarray(w_v.reshape(3, 128, 192), dtype=bf16),
        "w_o": np.ascontiguousarray(w_o.reshape(3, 64, 384), dtype=bf16),
    }


def _make_masks(bf16):
    m = np.zeros((4, 128, QT), dtype=np.float32)
    f = np.arange(QT)[None, :]
    p = np.arange(128)[:, None]
    for j in range(4):
        m[j] = (f - 128 * j >= p).astype(np.float32)
    return m.astype(bf16)


def kernel(x, w_attn, w_proj):
    import ml_dtypes
    bf16 = ml_dtypes.bfloat16

    x = np.asarray(x, dtype=np.float32)
    w_attn = np.asarray(w_attn, dtype=np.float32)
    w_proj = np.asarray(w_proj, dtype=np.float32)
    b, t, c = x.shape

    nc = _get_nc(t)
    masks = _make_masks(bf16)
    in_maps = []
    for core in range(8):
        im = _prep_core_inputs(x[core // 2], w_attn, w_proj, core % 2, bf16)
        im["masks"] = masks
        in_maps.append(im)

    res = run_bass_kernel_spmd(nc, in_maps, list(range(8)))
    out = np.empty((b, t, c), dtype=np.float32)
    for bb in range(b):
        out[bb] = res.results[2 * bb]["y"] + res.results[2 * bb + 1]["y"]
    return out


# revision 12
# speedup vs baseline: 1.1048x; 1.1048x over previous
"""Causal self-attention Trainium2 kernel (B=4, T=4096, C=384, H=6).

Sharding: 8 cores = 4 batches x 2 head-groups (3 heads each). Each core
computes y_partial = attn(x[b], heads hg) @ w_proj[rows of hg]; the host
sums the two partials per batch (the "all-reduce after c_proj" done on
host during unshard).
"""

import numpy as np
from contextlib import ExitStack

import concourse.bass as bass
import concourse.tile as tile
from concourse import mybir
from concourse.bass_utils import run_bass_kernel_spmd
from concourse.masks import make_identity
from concourse.vector_clock import ScopedClock

F32 = mybir.dt.float32
BF16 = mybir.dt.bfloat16
EXP = mybir.ActivationFunctionType.Exp
MULT = mybir.AluOpType.mult

B, T, C, H, D = 4, 4096, 384, 6, 64
HPC = 3            # heads per core
QT = 512           # q tile
KC = 128           # key chunk
SCALE = 1.0 / 8.0  # 1/sqrt(64)


# ---------------------------------------------------------------------------
# Workaround: neuronxcc CoreV3 rejects >2 sem waits on the Tile tail drain.
# Split the drain's waits into individual sync-engine wait instructions.
def _drain_and_barrier_split(self, tick_clock, wait_clock):
    nc = self.nc
    drain_inst = nc.sync.drain()
    wait_clock.add_sem_waits(
        drain_inst.ins, ScopedClock({None: tick_clock.global_clock})
    )
    si = drain_inst.ins.sync_info
    if si is not None and si.on_wait and len(si.on_wait) > 1:
        waits = list(si.on_wait)
        si.on_wait = []
        allocated = {h.name: h for h in self.sems.allocated().values()}
        for w in waits:
            h = allocated.get(w.ant_name)
            assert h is not None, f"no sem handle for drain wait {w.ant_name}"
            assert w.wait_mode == "sem-ge-imm", w.wait_mode
            nc.sync.wait_ge(h, w.wait_value)
    nc.all_engine_barrier()
    assert self.sems is not None
    popped = nc._tile_sem_poison_stack.pop()
    assert popped is self._sem_poison
    nc.clear_and_free_semaphores(list(self.sems.allocated().values()))
    nc.all_engine_barrier()


tile.TileContext._drain_and_barrier = _drain_and_barrier_split


MAX_WAITS = 1  # CoreV3 per-instruction sem-wait capacity (S3_LW holds only 1)


def _split_excess_waits(nc):
    """Hoist sem waits beyond MAX_WAITS onto same-engine NOPs inserted
    directly before the over-limit instruction (waits are order-free)."""
    for fn in nc.m.functions:
        for bb in fn.blocks:
            insts = list(bb.instructions)
            out = []
            changed = False
            for inst in insts:
                si = inst.sync_info
                if si is not None and si.on_wait and len(si.on_wait) > MAX_WAITS:
                    waits = list(si.on_wait)
                    excess, keep = waits[:-MAX_WAITS], waits[-MAX_WAITS:]
                    si.on_wait = keep
                    inst.sync_info = si
                    for i in range(0, len(excess), MAX_WAITS):
                        nop = mybir.InstNoOp(
                            name=f"{inst.name}-waitsplit-{i}", ins=[], outs=[]
                        )
                        nop.engine = inst.engine
                        nop.sync_info = mybir.SyncInfo(
                            on_wait=excess[i:i + MAX_WAITS], on_update=[]
                        )
                        nc.register_instruction(nop)
                        out.append(nop)
                    changed = True
                out.append(inst)
            if changed:
                bb.instructions = out
# ---------------------------------------------------------------------------


def build(t=T):
    nqt = t // QT          # q tiles
    nkc = t // KC          # key chunks
    ntb = t // QT          # token blocks for phase A (512 tokens each)

    nc = bass.Bass()
    x_d = nc.dram_tensor("x", [t, C], F32, kind="ExternalInput")
    wq01_d = nc.dram_tensor("w_q01", [3, 128, 128], BF16, kind="ExternalInput")
    wk01_d = nc.dram_tensor("w_k01", [3, 128, 128], BF16, kind="ExternalInput")
    wq2_d = nc.dram_tensor("w_q2", [3, 128, 64], BF16, kind="ExternalInput")
    wk2_d = nc.dram_tensor("w_k2", [3, 128, 64], BF16, kind="ExternalInput")
    wv_d = nc.dram_tensor("w_v", [3, 128, 192], BF16, kind="ExternalInput")
    wo_d = nc.dram_tensor("w_o", [3, 64, 384], BF16, kind="ExternalInput")
    mask_d = nc.dram_tensor("masks", [4, 128, QT], BF16, kind="ExternalInput")
    y_d = nc.dram_tensor("y", [t, C], F32, kind="ExternalOutput")
    # scratch for transposing the softmax denominator row into columns
    l_d = nc.dram_tensor("lscratch", [t // QT, 3, QT], F32)

    with tile.TileContext(nc) as tc, ExitStack() as ctx:
        persist = ctx.enter_context(tc.tile_pool(name="persist", bufs=1))

        # weights / masks / identity
        wq01 = persist.tile([128, 3, 128], BF16)
        wk01 = persist.tile([128, 3, 128], BF16)
        wq2 = persist.tile([128, 3, 64], BF16)
        wk2 = persist.tile([128, 3, 64], BF16)
        wv = persist.tile([128, 3, 192], BF16)
        wo = persist.tile([64, 3, 384], BF16)
        for c in range(3):
            nc.sync.dma_start(out=wq01[:, c, :], in_=wq01_d[c])
            nc.sync.dma_start(out=wk01[:, c, :], in_=wk01_d[c])
            nc.sync.dma_start(out=wq2[:, c, :], in_=wq2_d[c])
            nc.sync.dma_start(out=wk2[:, c, :], in_=wk2_d[c])
            nc.sync.dma_start(out=wv[:, c, :], in_=wv_d[c])
            nc.sync.dma_start(out=wo[:, c, :], in_=wo_d[c])
        masks = persist.tile([128, 4, QT], BF16)
        for j in range(4):
            nc.sync.dma_start(out=masks[:, j, :], in_=mask_d[j])
        ident = persist.tile([128, 128], F32)
        make_identity(nc, ident)


        # persistent activations (bf16)
        qT01 = persist.tile([128, t], BF16)   # rows 0:64 h0 qT, 64:128 h1 qT
        kT01 = persist.tile([128, t], BF16)
        qT2 = persist.tile([64, t], BF16)
        kT2 = persist.tile([64, t], BF16)
        vsb = persist.tile([128, nkc, 3, 65], BF16)  # [keys, chunk, head, d|one]
        nc.vector.memset(vsb[:, :, :, 64:65], 1.0)

        # ---------------- Phase A: qkv projection -------------------------
        with (
            tc.tile_pool(name="xin", bufs=3) as xin_p,
            tc.tile_pool(name="xt", bufs=2) as xt_p,
            tc.tile_pool(name="ps_tp", bufs=4, space="PSUM") as ps_tp,
            tc.tile_pool(name="ps_qk", bufs=2, space="PSUM") as ps_qk,
            tc.tile_pool(name="ps_v", bufs=2, space="PSUM") as ps_v,
        ):
            for tb in range(ntb):
                xt = xt_p.tile([128, 3, QT], BF16)  # xT chunk block
                for s in range(4):
                    xx = xin_p.tile([128, C], F32)
                    nc.sync.dma_start(
                        out=xx[:],
                        in_=x_d[(tb * 4 + s) * 128:(tb * 4 + s + 1) * 128, :],
                    )
                    for c in range(3):
                        tp = ps_tp.tile([128, 128], F32)
                        nc.tensor.transpose(
                            tp[:], xx[:, c * 128:(c + 1) * 128], ident[:]
                        )
                        nc.vector.tensor_copy(
                            xt[:, c, s * 128:(s + 1) * 128], tp[:]
                        )
                # q/k feature-transposed blocks
                for w_sb, m, dst in (
                    (wq01, 128, qT01),
                    (wk01, 128, kT01),
                    (wq2, 64, qT2),
                    (wk2, 64, kT2),
                ):
                    ps = ps_qk.tile([128, QT], F32)
                    for c in range(3):
                        nc.tensor.matmul(
                            ps[0:m, :], w_sb[:, c, 0:m], xt[:, c, :],
                            start=(c == 0), stop=(c == 2),
                        )
                    nc.vector.tensor_copy(
                        dst[0:m, tb * QT:(tb + 1) * QT], ps[0:m, :]
                    )
                # V natural
                for s in range(4):
                    psv = ps_v.tile([128, 3, 64], F32)
                    for c in range(3):
                        nc.tensor.matmul(
                            psv[:, :, :].rearrange("p h d -> p (h d)"),
                            xt[:, c, s * 128:(s + 1) * 128],
                            wv[:, c, :],
                            start=(c == 0), stop=(c == 2),
                        )
                    nc.vector.tensor_copy(
                        vsb[:, tb * 4 + s, :, 0:64], psv[:, :, :]
                    )

        # ---------------- Phase B: attention + c_proj ---------------------
        # Heads processed sequentially per q-tile: ps_s gets 3 double-bank
        # slots (deep S^T matmul pipelining ahead of the exp stream) and
        # att/y share a 2-slot single-bank pool: 3*2 + 2 = 8 PSUM banks.
        qk_src = [
            (kT01[0:64, :], qT01[0:64, :]),
            (kT01[64:128, :], qT01[64:128, :]),
            (kT2, qT2),
        ]
        with (
            tc.tile_pool(name="ps_s", bufs=3, space="PSUM") as ps_s,
            tc.tile_pool(name="ps_atty", bufs=2, space="PSUM") as ps_atty,
            tc.tile_pool(name="pth", bufs=3) as pth_p,
            tc.tile_pool(name="attn", bufs=6) as attn_p,
            tc.tile_pool(name="lrow", bufs=3) as lrow_p,
            tc.tile_pool(name="lcol", bufs=6) as lcol_p,
            tc.tile_pool(name="yout", bufs=3) as yout_p,
        ):
            for qt in range(nqt):
                nch = 4 * (qt + 1)      # causal key chunks for this q tile
                q0, q1 = qt * QT, (qt + 1) * QT

                attn_tiles = []
                linv_tiles = []
                for h in range(3):
                    kTh, qTh = qk_src[h]
                    pth = pth_p.tile([128, nkc // 2, 2, QT], BF16, tag="pth")
                    # S^T + exp, 2 key chunks per exp instruction
                    for g in range(nch // 2):
                        ss = ps_s.tile([128, 2, QT], F32, tag="ss")
                        for j in range(2):
                            ck = 2 * g + j
                            nc.tensor.matmul(
                                ss[:, j, :],
                                kTh[:, ck * KC:(ck + 1) * KC],
                                qTh[:, q0:q1],
                                start=True, stop=True,
                            )
                        nc.scalar.activation(
                            out=pth[:, g, :, :].rearrange("p j q -> p (j q)"),
                            in_=ss[:, :, :].rearrange("p j q -> p (j q)"),
                            func=EXP, scale=SCALE,
                        )
                    # causal masks on the 4 diagonal chunks
                    for j in range(4):
                        ck = 4 * qt + j
                        nc.vector.tensor_tensor(
                            out=pth[:, ck // 2, ck % 2, :],
                            in0=pth[:, ck // 2, ck % 2, :],
                            in1=masks[:, j, :], op=MULT,
                        )
                    # att^T accumulation: lhsT = [V_h | 1]  ->  [65, QT]
                    att = ps_atty.tile([65, QT], F32, tag="atty")
                    for ck in range(nch):
                        nc.tensor.matmul(
                            att[:], vsb[:, ck, h, :],
                            pth[:, ck // 2, ck % 2, :],
                            start=(ck == 0), stop=(ck == nch - 1),
                        )
                    # unnormalized att -> bf16 (c_proj lhsT); l row -> columns
                    at = attn_p.tile([64, QT], BF16, tag="attn")
                    attn_tiles.append(at)
                    nc.vector.tensor_copy(at[:], att[0:64, :])
                    lrow = lrow_p.tile([65, QT], F32, tag="lrow")
                    nc.vector.tensor_copy(lrow[64:65, :], att[64:65, :])
                    nc.sync.dma_start(out=l_d[qt, h], in_=lrow[64:65, :])
                    lcol = lcol_p.tile([128, 4], F32, tag="lcol")
                    nc.sync.dma_start(
                        out=lcol[:],
                        in_=l_d[qt, h].rearrange("(s p) -> p s", p=128),
                    )
                    linv = lcol_p.tile([128, 4], F32, tag="linv")
                    linv_tiles.append(linv)
                    nc.vector.reciprocal(linv[:], lcol[:])

                # c_proj: y[q0:q1] = sum_h (attn_h^T @ w_o[h]) / l_h
                for s in range(4):
                    ysb = yout_p.tile([128, C], F32, tag="ysb")
                    for h in range(3):
                        yp = ps_atty.tile([128, C], F32, tag="atty")
                        nc.tensor.matmul(
                            yp[:],
                            attn_tiles[h][:, s * 128:(s + 1) * 128],
                            wo[:, h, :],
                            start=True, stop=True,
                        )
                        sc = linv_tiles[h][:, s:s + 1]
                        if h == 0:
                            nc.vector.tensor_scalar(
                                out=ysb[:], in0=yp[:], scalar1=sc,
                                scalar2=None, op0=MULT,
                            )
                        else:
                            nc.vector.scalar_tensor_tensor(
                                out=ysb[:], in0=yp[:], scalar=sc, in1=ysb[:],
                                op0=MULT, op1=mybir.AluOpType.add,
                            )
                    nc.sync.dma_start(
                        out=y_d[q0 + s * 128:q0 + (s + 1) * 128, :], in_=ysb[:]
                    )

    _split_excess_waits(nc)
    nc.finalize()
    return nc


_NC_CACHE = {}


def _get_nc(t=T):
    if t not in _NC_CACHE:
        _NC_CACHE[t] = build(t)
    return _NC_CACHE[t]


def _prep_core_inputs(x_b, w_attn, w_proj, hg, bf16):
    """Host-side shard prep for one core: batch x_b, head group hg (0/1)."""
    h0 = 3 * hg
    q = w_attn[:, 0:C]
    k = w_attn[:, C:2 * C]
    v = w_attn[:, 2 * C:3 * C]
    qcols = lambda h: q[:, h * D:(h + 1) * D]
    kcols = lambda h: k[:, h * D:(h + 1) * D]
    w_q01 = np.concatenate([qcols(h0), qcols(h0 + 1)], axis=1)      # [384,128]
    w_k01 = np.concatenate([kcols(h0), kcols(h0 + 1)], axis=1)
    w_q2 = qcols(h0 + 2)                                            # [384,64]
    w_k2 = kcols(h0 + 2)
    w_v = v[:, h0 * D:(h0 + 3) * D]                                 # [384,192]
    w_o = w_proj[h0 * D:(h0 + 3) * D, :]                            # [192,384]
    return {
        "x": np.ascontiguousarray(x_b, dtype=np.float32),
        "w_q01": np.ascontiguousarray(w_q01.reshape(3, 128, 128), dtype=bf16),
        "w_k01": np.ascontiguousarray(w_k01.reshape(3, 128, 128), dtype=bf16),
        "w_q2": np.ascontiguousarray(w_q2.reshape(3, 128, 64), dtype=bf16),
        "w_k2": np.ascontiguousarray(w_k2.reshape(3, 128, 64), dtype=bf16),
        "w_v": np.ascontiguousarray(w_v.reshape(3, 128, 192), dtype=bf16),
        "w_o": np.ascontiguousarray(w_o.reshape(3, 64, 384), dtype=bf16),
    }


def _make_masks(bf16):
    m = np.zeros((4, 128, QT), dtype=np.float32)
    f = np.arange(QT)[None, :]
    p = np.arange(128)[:, None]
    for j in range(4):
        m[j] = (f - 128 * j >= p).astype(np.float32)
    return m.astype(bf16)


def kernel(x, w_attn, w_proj):
    import ml_dtypes
    bf16 = ml_dtypes.bfloat16

    x = np.asarray(x, dtype=np.float32)
    w_attn = np.asarray(w_attn, dtype=np.float32)
    w_proj = np.asarray(w_proj, dtype=np.float32)
    b, t, c = x.shape

    nc = _get_nc(t)
    masks = _make_masks(bf16)
    in_maps = []
    for core in range(8):
        im = _prep_core_inputs(x[core // 2], w_attn, w_proj, core % 2, bf16)
        im["masks"] = masks
        in_maps.append(im)

    res = run_bass_kernel_spmd(nc, in_maps, list(range(8)))
    out = np.empty((b, t, c), dtype=np.float32)
    for bb in range(b):
        out[bb] = res.results[2 * bb]["y"] + res.results[2 * bb + 1]["y"]
    return out
